# revision 21
# baseline (speedup 1.0000x reference)
"""Trainium2 Bass kernel for nn_CrossAttention_31791347925417.

Math (per batch b, stream tok in {x, blood} with weight W in {W1, W2}):
    kv = tok @ W.T ; k, v heads [H, N, D]
    ctx = softmax_d( SCALE * k_h^T v_h )          # [H, D, D], softmax over first D
    out_x = x_h @ ctx2_h ; out_b = blood_h @ ctx1_h

Gram trick: k_h^T v_h = W_k_h (tok^T tok) W_v_h^T with G = tok^T tok [C, C],
so the N=4096 contraction happens once per stream and everything downstream is
tiny [C,C]-scale work.

v3 notes:
- Tokens ship in BF16 (rel_l2 ~ 4e-3 vs the 2e-2 budget); W stays f32.
- All DRAM inputs are HOST-PREARRANGED into the exact SBUF tile layout, so
  every load is a direct [128, X] copy with 128 contiguous >=2KB descriptors
  (the v2 rearrange patterns cost ~38us of serial descriptor generation on
  the Sync engine).
- x additionally ships pre-TRANSPOSED (xT) from the host, eliminating its 128
  PE transposes + DVE drains; blood is transposed on-chip (its xT tiles are
  consumed immediately, so they stay in a small rotating buffer).
- G runs m-outer (one column-block row per sweep over all 32 n-tiles), so at
  most 2 G psum banks are live and each g row drains on scalar/vector behind
  the next sweep; mirrors slot between sweeps. Frees psum for psO bufs=4,
  which unblocks the out-matmul -> drain pipeline.
- Output DMA issues on the (otherwise idle) Sync engine, outputs are BF16 and
  upcast on the host.

Sharding: data-parallel over batch B=8 across the 8 cores; weights replicated.
Host pre-transposes W -> W.T [C, 2C] and folds SCALE into the k-half (exact,
SCALE = 0.125).
"""

import sys

if "/opt/trn_rl_repo" not in sys.path:
    sys.path.insert(0, "/opt/trn_rl_repo")

import ml_dtypes
import numpy as np

from concourse import bacc, masks, mybir, tile
from concourse.bass_utils import run_bass_kernel_spmd

B, N, C, H = 8, 4096, 512, 8
D = C // H
SCALE = D ** -0.5
P = 128
NBIG = N // 512          # 8 big row tiles (512 rows each)
NT = N // P              # 32 n-tiles
CB = C // P              # 4 column blocks == head pairs
F32 = mybir.dt.float32
F32R = mybir.dt.float32r
BF16 = mybir.dt.bfloat16
AX = mybir.AxisListType
ACT_EXP = mybir.ActivationFunctionType.Exp

# upper-triangle block schedule: row-block m computes cols [G_OFF[m], C)
G_OFF = [0, P, 2 * P, 3 * P]


def build_nc():
    nc = bacc.Bacc("TRN2", target_bir_lowering=False, debug=False)

    xb = nc.dram_tensor("xb", [P, NT * C], BF16, kind="ExternalInput").ap()
    bb = nc.dram_tensor("bb", [P, NT * C], BF16, kind="ExternalInput").ap()
    xtd = nc.dram_tensor("xtd", [P, CB * N], BF16, kind="ExternalInput").ap()
    w1t = nc.dram_tensor("w1t", [P, CB * 2 * C], F32R, kind="ExternalInput").ap()
    w2t = nc.dram_tensor("w2t", [P, CB * 2 * C], F32R, kind="ExternalInput").ap()
    # blocked transposed output layout: [kb, part(c within pair), pair, n-col]
    ox = nc.dram_tensor("oxT", [NBIG, P, CB, 512], BF16, kind="ExternalOutput").ap()
    ob = nc.dram_tensor("obT", [NBIG, P, CB, 512], BF16, kind="ExternalOutput").ap()

    with tile.TileContext(nc) as tc:
        _emit(nc, tc, xb, bb, xtd, w1t, w2t, ox, ob)

    nc.compile()
    return nc


def _emit(nc, tc, xb, bb, xtd, w1t, w2t, ox, ob):
    from contextlib import ExitStack

    ctx = ExitStack()
    with ctx:
        const = ctx.enter_context(tc.tile_pool(name="const", bufs=1))
        wpool = ctx.enter_context(tc.tile_pool(name="wpool", bufs=1))
        tokp = ctx.enter_context(tc.tile_pool(name="tokp", bufs=12))
        xtp = ctx.enter_context(tc.tile_pool(name="xtp", bufs=1))
        xtbp = ctx.enter_context(tc.tile_pool(name="xtbp", bufs=2))
        gqp = ctx.enter_context(tc.tile_pool(name="gqp", bufs=8))
        smallp = ctx.enter_context(tc.tile_pool(name="smallp", bufs=2))
        fpool = ctx.enter_context(tc.tile_pool(name="fpool", bufs=8))
        bdpool = ctx.enter_context(tc.tile_pool(name="bdpool", bufs=8))
        ostp = ctx.enter_context(tc.tile_pool(name="ostp", bufs=4))
        psG = ctx.enter_context(tc.tile_pool(name="psG", bufs=2, space="PSUM"))
        psT = ctx.enter_context(tc.tile_pool(name="psT", bufs=2, space="PSUM"))
        psO = ctx.enter_context(tc.tile_pool(name="psO", bufs=4, space="PSUM"))

        ident = const.tile([P, P], F32, tag="idf")
        masks.make_identity(nc, ident[:])
        ident_bf = const.tile([P, P], BF16, tag="idb")
        masks.make_identity(nc, ident_bf[:])

        # weights: chunk j (c-rows 128j..128j+128) lives at cols [j*2C, (j+1)*2C)
        w_x = wpool.tile([P, CB * 2 * C], F32R, tag="wx")
        w_b = wpool.tile([P, CB * 2 * C], F32R, tag="wb")

        def wchunk(w, j):
            return w[:, j * 2 * C:(j + 1) * 2 * C]

        # pre-transposed x from DRAM: pair block m at cols [m*N, (m+1)*N)
        xT_x = xtp.tile([P, CB * N], BF16, tag="xtx")

        def emit_loads(tok_dram, eng, tag, split_first=False, only=None):
            toks = []
            for kb in (range(NBIG) if only is None else only):
                tokb = tokp.tile([P, 4 * C], BF16, tag=tag, name=f"{tag}{kb}")
                if kb == 0 and split_first:
                    for sub in range(4):
                        eng.dma_start(
                            tokb[:, sub * C:(sub + 1) * C],
                            tok_dram[:, sub * C:(sub + 1) * C])
                else:
                    eng.dma_start(
                        tokb[:], tok_dram[:, kb * 4 * C:(kb + 1) * 4 * C])
                toks.append(tokb)
            return toks

        def emit_G_sweep(gps_list, toks, ms, per_tile=None):
            for kb in range(NBIG):
                for sub in range(4):
                    k = kb * 4 + sub
                    sb = toks[kb][:, sub * C:(sub + 1) * C]
                    for gps, m in zip(gps_list, ms):
                        o = G_OFF[m]
                        nc.tensor.matmul(
                            gps[:, o:C], sb[:, m * P:(m + 1) * P], sb[:, o:C],
                            start=(k == 0), stop=(k == NT - 1),
                        )
                    if per_tile is not None:
                        per_tile(kb, sub, sb)

        def g_drain(g_sb, gps, m):
            o = G_OFF[m]
            if m % 2:
                nc.scalar.copy(g_sb[m][:, o:C], gps[:, o:C])
            else:
                nc.vector.tensor_copy(g_sb[m][:, o:C], gps[:, o:C])

        def emit_mirrors(g_sb, j):
            # lower blocks (i,j), i > j, from (j,i)^T ; g_sb[j] already drained
            for i in range(j + 1, CB):
                mps = psT.tile([P, P], F32, tag="t", name="mps")
                nc.tensor.transpose(
                    mps[:], g_sb[j][:, i * P:(i + 1) * P].bitcast(F32),
                    ident[:],
                )
                if i % 2:
                    nc.scalar.copy(g_sb[i][:, j * P:(j + 1) * P], mps[:])
                else:
                    nc.vector.tensor_copy(g_sb[i][:, j * P:(j + 1) * P], mps[:])

        def emit_T_tile(xT, xt_col, sb, alt=0):
            tps = psT.tile([P, C], BF16, tag="t", name="tps")
            for m in range(CB):
                nc.tensor.transpose(
                    tps[:, m * P:(m + 1) * P], sb[:, m * P:(m + 1) * P],
                    ident_bf[:],
                )
            dst = xT[:].rearrange("p (m n) -> p m n", m=CB)[
                :, :, xt_col:xt_col + P]
            src = tps[:].rearrange("p (m n) -> p m n", m=CB)
            if alt % 2:
                nc.scalar.copy(dst, src)
            else:
                nc.vector.tensor_copy(dst, src)

        def out_chunk(xT, xt_stride, xt_base, BDs, kb, od):
            """outT for 512 n-cols (tile-group kb): per pair p one matmul
            [c-block p, 512 n]; each psum drains as two half-col copies on
            scalar+vector (halves the psum hold time), two half-DMAs."""
            for h in range(2):
                ost = ostp.tile([P, 2 * 512], BF16, tag="ost", name="ost")
                for pp_ in range(2):
                    p = 2 * h + pp_
                    ops = psO.tile([P, 512], F32, tag="o", name=f"ops{p}")
                    nc.tensor.matmul(
                        ops[:], BDs[p][:],
                        xT[:, p * xt_stride + xt_base:
                           p * xt_stride + xt_base + 512],
                        start=True, stop=True,
                    )
                    o0 = pp_ * 512
                    nc.vector.tensor_copy(ost[:, o0:o0 + 256], ops[:, 0:256])
                    nc.scalar.copy(ost[:, o0 + 256:o0 + 512], ops[:, 256:512])
                nc.sync.dma_start(od[kb][:, 2 * h:2 * h + 2, :], ost[:])

        def emit_chain_mid(g_sb, w):
            """After all g rows drained+mirrored: Q -> ctx logits -> softmax.
            Returns normalized prob tiles fp (sbuf, bf16); BD transposes are
            deferred so the PE stream can run ahead of softmax."""
            q_sb = [None] * CB
            for i in reversed(range(CB)):
                qp = psO.tile([P, C], F32, tag="o", name=f"qp{i}")
                for j in range(CB):
                    nc.tensor.matmul(
                        qp[:], g_sb[j][:, i * P:(i + 1) * P],
                        wchunk(w, j)[:, 0:C], start=(j == 0), stop=(j == 3),
                    )
                q = gqp.tile([P, C], F32R, tag="gq", name=f"q{i}")
                if i % 2:
                    nc.scalar.copy(q[:], qp[:])
                else:
                    nc.vector.tensor_copy(q[:], qp[:])
                q_sb[i] = q

            fps = []
            for p in range(CB):
                # 256-wide moving window keeps f32r at 1 cyc/row; the diagonal
                # block we need sits at col offset 0 (p<3) or 128 (p=3)
                lo = p * P if p < 3 else 2 * P
                coff = 0 if p < 3 else P
                cps = psO.tile([P, C], F32, tag="o", name=f"cps{p}")
                for j in range(CB):
                    nc.tensor.matmul(
                        cps[:, 0:2 * P],
                        wchunk(w, j)[:, C + p * P:C + (p + 1) * P],
                        q_sb[j][:, lo:lo + 2 * P],
                        start=(j == 0), stop=(j == 3),
                    )
                nm = smallp.tile([P, 1], F32, tag="nm", name="nm")
                sm = smallp.tile([P, 1], F32, tag="sm", name="sm")
                rv = smallp.tile([P, 1], F32, tag="rv", name="rv")
                pp = smallp.tile([P, D], F32, tag="pp", name="pp")
                fp = fpool.tile([P, P], BF16, tag="F", name="fp")
                nc.gpsimd.memset(fp[:], 0.0)
                for dd in range(2):
                    s0 = slice(dd * D, (dd + 1) * D)
                    sc = slice(coff + dd * D, coff + (dd + 1) * D)
                    blk = cps[s0, sc]
                    nc.vector.reduce_max(nm[s0, :], blk, axis=AX.X, negate=True)
                    nc.scalar.activation(
                        pp[s0, :], blk, ACT_EXP, bias=nm[s0, :], scale=1.0,
                        accum_out=sm[s0, :],
                    )
                nc.vector.reciprocal(rv[:], sm[:])
                for dd in range(2):
                    s0 = slice(dd * D, (dd + 1) * D)
                    nc.vector.tensor_scalar_mul(fp[s0, s0], pp[s0, :], rv[s0, :])
                fps.append(fp)
            return fps

        def emit_bd(fps):
            BDs = []
            for p in range(CB):
                bps = psT.tile([P, P], BF16, tag="t", name="bps")
                nc.tensor.transpose(bps[:], fps[p][:], ident_bf[:])
                bd = bdpool.tile([P, P], BF16, tag="bd", name=f"bd{p}")
                nc.vector.tensor_copy(bd[:], bps[:])
                BDs.append(bd)
            return BDs

        def emit_gram(toks, g_sb, per_tile=None):
            """G sweeps: m=0+1 interleaved per tile (paces with the token DMA),
            then m=2 and m=3; drains/mirrors pipeline behind later sweeps."""
            gps0 = psG.tile([P, C], F32, tag="g", name="gps0")
            gps1 = psG.tile([P, C], F32, tag="g", name="gps1")
            emit_G_sweep([gps0, gps1], toks, [0, 1], per_tile)
            g_drain(g_sb, gps0, 0)
            g_drain(g_sb, gps1, 1)
            gps2 = psO.tile([P, C], F32, tag="o", name="gps2")
            emit_G_sweep([gps2], toks, [2])
            emit_mirrors(g_sb, 0)
            emit_mirrors(g_sb, 1)
            g_drain(g_sb, gps2, 2)
            gps3 = psG.tile([P, C], F32, tag="g", name="gps3")
            emit_G_sweep([gps3], toks, [3])
            emit_mirrors(g_sb, 2)
            g_drain(g_sb, gps3, 3)

        # ---- schedule ----
        # issue order = sync-queue order: x first (consumed immediately),
        # then the first blood chunks, weights, rest of blood. xtd (only
        # needed in phase C) is issued mid-phase-B so it never competes.
        toks_x = emit_loads(xb, nc.sync, "tx", split_first=True)
        toks_b = emit_loads(bb, nc.sync, "tb", only=range(0, 2))
        nc.sync.dma_start(w_x[:], w1t[:, :])
        nc.sync.dma_start(w_b[:], w2t[:, :])
        toks_b += emit_loads(bb, nc.sync, "tb", only=range(2, NBIG))

        # phase A: G_x sweeps (no transposes; xT_x ships from the host)
        g1_sb = [gqp.tile([P, C], F32R, tag="gq", name=f"g1_{m}")
                 for m in range(CB)]
        emit_gram(toks_x, g1_sb)
        f1 = emit_chain_mid(g1_sb, w_x)
        # xT_x load: issued on scalar (hwdge) once the input burst is over,
        # arrives long before the output phase needs it
        for m in range(CB):
            nc.scalar.dma_start(
                xT_x[:, m * N:(m + 1) * N], xtd[:, m * N:(m + 1) * N])

        # phase B: G_b sweeps with T_b + out_b pipelined per tile in the
        # m0/m1 sweep (PE work packed into the DMA-bound input window);
        # out_b lags one kb so PE never waits on softmax-1 or the drains
        g2_sb = [gqp.tile([P, C], F32R, tag="gq", name=f"g2_{m}")
                 for m in range(CB)]
        bstate = {"bd1": None}
        xtb_tiles = []

        def b_per_tile(kb, sub, sb):
            if sub == 0:
                xtb = xtbp.tile([P, CB * 512], BF16, tag="xtb", name="xtb")
                xtb_tiles.append(xtb)
            emit_T_tile(xtb_tiles[kb], sub * P, sb, alt=kb * 4 + sub)
            if sub == 3:
                if kb == 0:
                    bstate["bd1"] = emit_bd(f1)
                elif kb < NBIG - 1:
                    out_chunk(xtb_tiles[kb - 1], 512, 0,
                              bstate["bd1"], kb - 1, ob)

        emit_gram(toks_b, g2_sb, b_per_tile)
        # two reserved out_b chunks cover the chain-B softmax on the PE side
        out_chunk(xtb_tiles[NBIG - 2], 512, 0, bstate["bd1"], NBIG - 2, ob)
        f2 = emit_chain_mid(g2_sb, w_b)
        out_chunk(xtb_tiles[NBIG - 1], 512, 0, bstate["bd1"], NBIG - 1, ob)
        bd2 = emit_bd(f2)

        # phase C: out_x from the DMA-loaded xT_x
        for kb in range(NBIG):
            out_chunk(xT_x, N, kb * 512, bd2, kb, ox)


_NC_CACHE = None


def _get_nc():
    global _NC_CACHE
    if _NC_CACHE is None:
        _NC_CACHE = build_nc()
    return _NC_CACHE


def _prep_inputs(x, blood, W1, W2):
    x16 = np.asarray(x, dtype=np.float32).astype(ml_dtypes.bfloat16)
    b16 = np.asarray(blood, dtype=np.float32).astype(ml_dtypes.bfloat16)
    w1t = np.ascontiguousarray(np.asarray(W1, dtype=np.float32).T)
    w2t = np.ascontiguousarray(np.asarray(W2, dtype=np.float32).T)
    w1t[:, :C] *= SCALE  # fold softmax scale into the k-projection (exact: 2^-3)
    w2t[:, :C] *= SCALE

    def tok_layout(a):  # [N, C] -> [P, NT*C], partition-major tile layout
        return np.ascontiguousarray(
            a.reshape(NT, P, C).transpose(1, 0, 2).reshape(P, NT * C))

    def tokT_layout(a):  # [N, C] -> [P, CB*N], transposed tile layout
        return np.ascontiguousarray(
            a.T.reshape(CB, P, N).transpose(1, 0, 2).reshape(P, CB * N))

    def w_layout(a):  # [C, 2C] -> [P, CB*2C]
        return np.ascontiguousarray(
            a.reshape(CB, P, 2 * C).transpose(1, 0, 2).reshape(P, CB * 2 * C))

    w1l, w2l = w_layout(w1t), w_layout(w2t)
    return [
        {"xb": tok_layout(x16[b]), "bb": tok_layout(b16[b]),
         "xtd": tokT_layout(x16[b]), "w1t": w1l, "w2t": w2l}
        for b in range(B)
    ]


def _unshuffle(arr):
    """[NBIG, P, CB, 512] blocked-transposed bf16 -> [N, C] natural f32."""
    # arr[kb, part, p, col] = out[kb*512 + col, p*128 + part]
    return np.ascontiguousarray(
        arr.transpose(0, 3, 2, 1).reshape(N, C).astype(np.float32))


def kernel(x, blood, W1, W2, trace=False):
    nc = _get_nc()
    in_maps = _prep_inputs(x, blood, W1, W2)
    res = run_bass_kernel_spmd(nc, in_maps, core_ids=list(range(B)), trace=trace)
    out_x = np.stack([_unshuffle(res.results[b]["oxT"]) for b in range(B)])
    out_b = np.stack([_unshuffle(res.results[b]["obT"]) for b in range(B)])
    if trace:
        kernel.last_results = res
    return (out_x, out_b)
